# revision 9
# baseline (speedup 1.0000x reference)
"""Single-head causal attention (B=128, T=512, C=256, H=64) on 8 trn2 cores.

Data-parallel: 16 batches per core. x is transposed on the host to [B, C, T]
so the kernel loads xT directly (SWDGE fp32->bf16 cast, quad-batched): no
on-chip transpose, no psum->sbuf xT copy.

Per batch:
  [qT|kT] pair projections (batch j of a pair on partitions 64j..64j+63)
  simT[s,t] = kT-slice^T @ qT, all four s-chunks packed into one PSUM
  region (si0@0, si1@512, si3@896, si2@1024 floats; each matmul output
  stays inside one 2KB bank) so a single ACT instruction does exp over
  all 1280 columns. Causal diag masking accumulates -240 into the diag
  blocks via fp8e4 DoubleRow matmuls (half the cost of bf16 matmuls).
  v = xT^T @ Wv (natural [s,h] layout), ones column appended for rowsum.
  out_unnorm[t,h], rowsum[t] = pT^T @ [v|1];  out = out_unnorm * recip
  -> bf16 store; host upcasts to fp32.

Emission is software-pipelined with a one-batch skew: AV/normalize/store of
batch b-1 are emitted after sim of batch b so the PE does not block on exp.
"""
import numpy as np
import ml_dtypes

B, T, C, H = 128, 512, 256, 64
N_CORES = 8
BL = B // N_CORES          # batches per core
TC = T // 128              # 4 t-chunks
CS = C // 128              # 2 c-subtiles
INV_SQRT_H = 1.0 / np.sqrt(H)
QUAD = 4                   # batches per SWDGE load
# packed psim float offsets per s-chunk (each matmul output within one bank)
OFF = {0: 0, 1: 512, 2: 1024, 3: 896}
MASK_NEG = -240.0          # fp8e4 (trn e4m3) max normal


def _build_program():
    import concourse.tile as tile
    from concourse import bacc, mybir

    dt = mybir.dt
    DR = mybir.MatmulPerfMode.DoubleRow
    nc = bacc.Bacc("TRN2", target_bir_lowering=False, debug=False,
                   enable_asserts=False, num_devices=N_CORES)

    x_d = nc.dram_tensor("xt", [BL, C, T], dt.float32,
                         kind="ExternalInput").ap()
    wq_d = nc.dram_tensor("wq", [CS, 128, H], dt.bfloat16,
                          kind="ExternalInput").ap()
    wk_d = nc.dram_tensor("wk", [CS, 128, H], dt.bfloat16,
                          kind="ExternalInput").ap()
    wv_d = nc.dram_tensor("wv", [CS, 128, H], dt.bfloat16,
                          kind="ExternalInput").ap()
    i2_d = nc.dram_tensor("i2", [128, 2, 128], dt.float8e4,
                          kind="ExternalInput").ap()
    mn2_d = nc.dram_tensor("mn2", [128, 2, 128], dt.float8e4,
                           kind="ExternalInput").ap()
    out_d = nc.dram_tensor("out", [BL, T, H], dt.bfloat16,
                           kind="ExternalOutput").ap()

    with tile.TileContext(nc) as tc:
        from contextlib import ExitStack
        ctx = ExitStack()
        with ctx:
            consts = ctx.enter_context(tc.tile_pool(name="consts", bufs=1))
            sb_x = ctx.enter_context(tc.tile_pool(name="sb_x", bufs=2))
            sb_qk = ctx.enter_context(tc.tile_pool(name="sb_qk", bufs=3))
            sb_p = ctx.enter_context(tc.tile_pool(name="sb_p", bufs=3))
            sb_v = ctx.enter_context(tc.tile_pool(name="sb_v", bufs=3))
            sb_o = ctx.enter_context(tc.tile_pool(name="sb_o", bufs=6))
            ps_qk = ctx.enter_context(tc.tile_pool(name="ps_qk", bufs=1,
                                                   space="PSUM"))
            ps_sim = ctx.enter_context(tc.tile_pool(name="ps_sim", bufs=1,
                                                    space="PSUM"))
            ps_v = ctx.enter_context(tc.tile_pool(name="ps_v", bufs=1,
                                                  space="PSUM"))
            ps_av = ctx.enter_context(tc.tile_pool(name="ps_av", bufs=1,
                                                   space="PSUM"))

            wq_sb = consts.tile([128, CS, H], dt.bfloat16)
            nc.sync.dma_start(wq_sb[:], wq_d.rearrange("cs p h -> p cs h"))
            wk_sb = consts.tile([128, CS, H], dt.bfloat16)
            nc.sync.dma_start(wk_sb[:], wk_d.rearrange("cs p h -> p cs h"))
            wv_sb = consts.tile([128, CS, H], dt.bfloat16)
            nc.sync.dma_start(wv_sb[:], wv_d.rearrange("cs p h -> p cs h"))
            i2_sb = consts.tile([128, 2, 128], dt.float8e4)
            nc.sync.dma_start(i2_sb[:], i2_d)
            mn2_sb = consts.tile([128, 2, 128], dt.float8e4)
            nc.sync.dma_start(mn2_sb[:], mn2_d)

            xt_tiles = {}
            qk_tiles = {}
            pT_tiles = {}
            v1_tiles = {}
            pav_tiles = {}

            def emit_load(quad):
                xt4 = sb_x.tile([128, QUAD, CS, T], dt.bfloat16,
                                name=f"xt{quad}", tag="xt")
                nc.gpsimd.dma_start(
                    xt4[:], x_d[QUAD * quad:QUAD * (quad + 1)].rearrange(
                        "b (cs p) t -> p b cs t", p=128))
                for j in range(QUAD):
                    xt_tiles[QUAD * quad + j] = xt4[:, j]

            def emit_proj(pb):
                # pair projections: batch j on psum partitions 64j..64j+63
                pq = ps_qk.tile([128, T], dt.float32, name=f"pq{pb}",
                                tag="pq")
                pk = ps_qk.tile([128, T], dt.float32, name=f"pk{pb}",
                                tag="pk")
                for j in range(2):
                    xt = xt_tiles[2 * pb + j]
                    for w_sb, pt in ((wq_sb, pq), (wk_sb, pk)):
                        for cc in range(CS):
                            nc.tensor.matmul(
                                pt[64 * j:64 * (j + 1), :],
                                w_sb[:, cc, :], xt[:, cc, :],
                                start=(cc == 0), stop=(cc == CS - 1),
                                tile_position=(0, 64 * j))
                qk = sb_qk.tile([128, 2, T], dt.bfloat16, name=f"qk{pb}",
                                tag="qk")
                nc.vector.tensor_copy(qk[:, 0, :], pq[:])
                nc.vector.tensor_copy(qk[:, 1, :], pk[:])
                qk_tiles[2 * pb] = qk
                qk_tiles[2 * pb + 1] = qk

            def emit_v(b):
                xt = xt_tiles[b]
                pv = ps_v.tile([128, TC, 128], dt.float32, name=f"pv{b}",
                               tag="pv")
                for sc in range(TC):
                    for cc in range(CS):
                        nc.tensor.matmul(
                            pv[:, sc, 0:H],
                            xt[:, cc, 128 * sc:128 * (sc + 1)],
                            wv_sb[:, cc, :],
                            start=(cc == 0), stop=(cc == CS - 1))
                v1 = sb_v.tile([128, TC, H + 1], dt.bfloat16,
                               name=f"v1{b}", tag="v1")
                nc.vector.tensor_copy(v1[:, :, 0:H], pv[:, :, 0:H])
                nc.gpsimd.memset(v1[:, :, H:H + 1], 1.0)
                v1_tiles[b] = v1

            def emit_sim_exp(b):
                j = b % 2
                qk = qk_tiles[b]
                qT = qk[64 * j:64 * (j + 1), 0, :]
                kT = qk[64 * j:64 * (j + 1), 1, :]
                psim = ps_sim.tile([128, 1536], dt.float32,
                                   name=f"psim{b}", tag="psim")
                pT = sb_p.tile([128, 1280], dt.bfloat16, name=f"pT{b}",
                               tag="pT")
                for si in range(TC):
                    n_si = T - 128 * si
                    off = OFF[si]
                    nc.tensor.matmul(
                        psim[:, off:off + n_si],
                        kT[:, 128 * si:128 * (si + 1)],
                        qT[:, 128 * si:T],
                        start=True, stop=False,
                        tile_position=(64 * j, 0),
                        skip_group_check=True)
                    nc.tensor.matmul(
                        psim[:, off:off + 128],
                        i2_sb[:], mn2_sb[:],
                        start=False, stop=True,
                        perf_mode=DR,
                        skip_group_check=True)
                    if si == 0:
                        # exp over psim bank 0 overlaps the remaining sims;
                        # frees the bank for the next batch's si0 early.
                        nc.scalar.activation(
                            pT[:, 0:512], psim[:, 0:512],
                            mybir.ActivationFunctionType.Exp,
                            scale=float(INV_SQRT_H))
                nc.scalar.activation(
                    pT[:, 512:1280], psim[:, 512:1280],
                    mybir.ActivationFunctionType.Exp,
                    scale=float(INV_SQRT_H))
                pT_tiles[b] = pT

            def emit_av(b):
                pT = pT_tiles[b]
                v1 = v1_tiles[b]
                pav = ps_av.tile([128, TC, 128], dt.float32,
                                 name=f"pav{b}", tag="pav")
                for ci in range(TC):
                    for si in range(ci + 1):
                        nc.tensor.matmul(
                            pav[:, ci, 0:H + 1],
                            pT[:, OFF[si] + 128 * (ci - si):
                               OFF[si] + 128 * (ci - si) + 128],
                            v1[:, si, :],
                            start=(si == 0), stop=(si == ci))
                pav_tiles[b] = pav

            def emit_out(b):
                pav = pav_tiles[b]
                rec = sb_o.tile([128, TC], dt.float32, name=f"rec{b}",
                                tag="rec")
                nc.vector.reciprocal(rec[:], pav[:, :, H])
                osb = sb_o.tile([128, TC, H], dt.bfloat16,
                                name=f"osb{b}", tag="osb")
                nc.vector.tensor_mul(
                    out=osb[:],
                    in0=pav[:, :, 0:H],
                    in1=rec[:, :, None].to_broadcast([128, TC, H]))
                nc.sync.dma_start(
                    out_d[b].rearrange("(tc p) h -> p tc h", p=128),
                    osb[:])

            # software-pipelined emission: AV/out of batch b-1 after sim of b
            for b in range(BL + 1):
                if b < BL:
                    if b % QUAD == 0:
                        emit_load(b // QUAD)
                    if b % 2 == 0:
                        emit_proj(b // 2)
                    emit_v(b)
                    emit_sim_exp(b)
                if b >= 1:
                    emit_av(b - 1)
                    emit_out(b - 1)

    nc.compile()
    return nc


_CACHED = None


def _get_program():
    global _CACHED
    if _CACHED is None:
        _CACHED = _build_program()
    return _CACHED


def _host_inputs(Wq, Wk, Wv):
    bf16 = ml_dtypes.bfloat16
    fp8 = ml_dtypes.float8_e4m3
    idm = np.eye(128, dtype=np.float32)
    i2 = np.zeros((128, 2, 128), np.float32)
    i2[:, 0, :] = idm
    mn2 = np.zeros((128, 2, 128), np.float32)
    mn2[:, 0, :] = np.where(
        np.arange(128)[None, :] < np.arange(128)[:, None],
        np.float32(MASK_NEG), np.float32(0))
    consts = {
        "wq": np.ascontiguousarray(
            np.asarray(Wq, np.float32).reshape(CS, 128, H)).astype(bf16),
        "wk": np.ascontiguousarray(
            np.asarray(Wk, np.float32).reshape(CS, 128, H)).astype(bf16),
        "wv": np.ascontiguousarray(
            np.asarray(Wv, np.float32).reshape(CS, 128, H)).astype(bf16),
        "i2": i2.astype(fp8),
        "mn2": mn2.astype(fp8),
    }
    return consts


def kernel(input_embeddings, Wq, Wk, Wv):
    from concourse.bass_utils import run_bass_kernel_spmd

    x = np.asarray(input_embeddings, np.float32)
    xt = np.ascontiguousarray(x.transpose(0, 2, 1))   # [B, C, T]
    nc = _get_program()
    consts = _host_inputs(Wq, Wk, Wv)
    in_maps = []
    for c in range(N_CORES):
        m = {"xt": xt[c * BL:(c + 1) * BL]}
        m.update(consts)
        in_maps.append(m)
    res = run_bass_kernel_spmd(nc, in_maps, core_ids=list(range(N_CORES)))
    out = np.concatenate([res.results[c]["out"] for c in range(N_CORES)],
                         axis=0)
    return out.astype(np.float32)


if __name__ == "__main__":
    rng = np.random.default_rng(0)
    x = rng.standard_normal((B, T, C)).astype(np.float32)
    wq = (rng.standard_normal((C, H)) / 16).astype(np.float32)
    wk = (rng.standard_normal((C, H)) / 16).astype(np.float32)
    wv = (rng.standard_normal((C, H)) / 16).astype(np.float32)
    out = kernel(x, wq, wk, wv)
    print("out", out.shape, out.dtype)
